# revision 46
# baseline (speedup 1.0000x reference)
"""Additive attention (tanh-score) via separable sin-basis, TRN2 x8.

scores[b,q,k] = sum_h w_v[h] * tanh(qp[b,q,h] + kp[b,k,h])
              ~ sum_h w_v[h] * [ h(a) + sum_r c_r sin(w_r a + s_r)sin(w'_r b + s'_r) ]

Pure-a term is softmax-invariant => dropped. Each sin atom is evaluated as
  u = t - round(t),  t = (w x + s) / 2pi     (ONE fused custom DVE op,
                                              fp32 magic-constant rounding)
  atom = Sin(2pi * u)                        (ACT, scale/bias identical for
                                              every atom => q|k concatenated
                                              into one ACT call per slot)
The wv*c fold of the k-side runs on GpSimd (otherwise idle). Scores
accumulate in PSUM across slots; a mask pseudo-slot (ones lhsT x mask rows)
closes the groups. Softmax: fused-exp ACT with row-sum accumulator, PE
transposes, attn @ values on PE, 1/rowsum applied on the output tiles.

Sharding: 2 batches per core (big+small valid_len paired), baked LP0/LP1.
"""

import os
import numpy as np
import ml_dtypes

_NCORES = 8
BF16 = ml_dtypes.bfloat16
FP16 = np.float16

# (w_q, s_q, w_k, s_k, c): tanh(a+b) ~ sum c*sin(w_q a + s_q)*sin(w_k b + s_k)
SLOTS6 = [
    (-0.4400441419, 1.5829518017, -0.4404289683, -3.1577059831, 1.1875448050),
    (-2.5000000000, 0.2873262163, -2.5000000000, -1.7679537402, 0.0454728641),
    (1.5184027148, -3.2031135619, -1.5172863264, -1.6228271159, 0.1612171955),
    (-2.4691279788, -1.3872222109, 2.4692423942, 0.2646449817, -0.0516913971),
    (1.3705130957, -1.6155418902, 1.3700050256, 0.0533663718, -0.2331078125),
    (-0.7178178262, -3.1153805677, -0.7182280272, 1.5504013584, 0.5396772381),
]
SLOTS5 = [
    (-0.4531, -1.5708, -0.4534, 3.1416, -1.18236),
    (-2.5446, -1.5708, -2.5450, -0.0000, 0.04739),
    (0.9812, -0.0000, 0.9853, -1.5708, -0.45954),
    (-1.4115, -1.5708, -1.4112, -0.0000, 0.22485),
    (-2.1211, -3.1416, 2.1176, -1.5708, -0.09258),
]
SLOTS = SLOTS6 if os.environ.get("KERNEL_SLOTS6") == "1" else SLOTS5

NQ = 256
D = 256
H = 256
DV = 256
NDC = D // 128
NHC = H // 128
MAGIC = 12582912.0  # 1.5 * 2**23: fp32 add/sub rounds to nearest integer
TWO_PI = 6.283185307179586


def _register_ntff_hook():
    import sys, types
    try:
        from antenv.axon_hooks import get_axon_ntff_profile_hook  # noqa: F401
        return
    except ImportError:
        pass
    try:
        import trn_agent_boot.trn_boot as tb
        mod = types.ModuleType("antenv.axon_hooks")
        hook = tb._ntff_profile_via_ctypes("/opt/axon/libaxon_pjrt.so")
        mod.get_axon_ntff_profile_hook = lambda: hook
        mod.set_axon_ntff_profile_hook = lambda h: None
        sys.modules["antenv.axon_hooks"] = mod
    except Exception:
        pass


def _register_rr_op():
    """Custom DVE op: u = t - round(t), t = Src0*C0 + C1 (C2 = MAGIC)."""
    import concourse.dve_ops as dops
    from concourse.dve_spec import Spec, Src0, C0, C1, C2, lower, _has_src1
    from concourse.dve_uop import DveOpSpec

    for o in dops.OPS:
        if o.name == "SIN_RR_ANT":
            return o
    t = Src0 * C0 + C1
    n = (t + C2) - C2
    spec = Spec(
        body=t - n,
        reference=lambda in0, in1, s0, s1, imm2: (
            lambda tt: (tt - (np.float32(np.float32(tt) + np.float32(imm2))
                             - np.float32(imm2))).astype(np.float32)
        )(np.float32(in0) * np.float32(s0) + np.float32(s1)),
    )
    row = dops._CUSTOM_DVE_ROW_BASE + len(dops.OPS)
    assert row < 0x20
    shas = {}
    for ver in ("v3", "v4"):
        uops = lower(spec, ver=ver)
        s = DveOpSpec(name="SIN_RR_ANT", opcode=row, uops=uops,
                      rd1_en=_has_src1(spec))
        shas[ver] = s.sha(ver)
    op = dops.DveOp("SIN_RR_ANT", spec, subdim=False, uops_sha=shas)
    dops.OPS.append(op)
    dops._SUB_OPCODE_FOR_NAME[op.name] = row
    dops.CUSTOM_DVE_SPECS[op.name] = spec
    return op


def _build_graph(LP0, LP1):
    import concourse.bass as bass
    import concourse.tile as tile
    from concourse import bacc, mybir, masks

    RR = _register_rr_op()

    f32 = mybir.dt.float32
    bf16 = mybir.dt.bfloat16
    fp16 = mybir.dt.float16
    AF = mybir.ActivationFunctionType
    ALU = mybir.AluOpType
    PSUM = bass.MemorySpace.PSUM

    LPT = LP0 + LP1
    LPs = (LP0, LP1)
    OFS = (0, LP0)
    NKC = ((LP0 + 127) // 128, (LP1 + 127) // 128)
    KCW = [[min(128, LPs[s] - 128 * c) for c in range(NKC[s])] for s in (0, 1)]
    QW = 2 * NQ                  # 512 q-cols (both batch slots)
    W = QW + LPT                 # per-hc projection width

    nc = bacc.Bacc("TRN2", target_bir_lowering=False, debug=False,
                   num_devices=_NCORES)

    d1_ds = [nc.dram_tensor(f"wq_qT{dc}", (128, 256 + QW), fp16,
                            kind="ExternalInput") for dc in range(NDC)]
    d2_ds = [nc.dram_tensor(f"wk_kT{dc}", (128, 256 + LPT), fp16,
                            kind="ExternalInput") for dc in range(NDC)]
    d3_d = nc.dram_tensor("vm", (128, 4 * DV + LPT), fp16,
                          kind="ExternalInput")
    wv_d = nc.dram_tensor("wvp", (128, NHC), f32, kind="ExternalInput")
    out_d = nc.dram_tensor("out", (2, NQ, DV), bf16, kind="ExternalOutput")

    with tile.TileContext(nc) as tc:
        with (
            tc.tile_pool(name="const", bufs=1) as constp,
            tc.tile_pool(name="basis", bufs=1) as basisp,
            tc.tile_pool(name="uw", bufs=4) as uwp,
            tc.tile_pool(name="atw", bufs=3) as atwp,
            tc.tile_pool(name="vtw", bufs=3) as vtwp,
            tc.tile_pool(name="epi", bufs=1) as epip,
            tc.tile_pool(name="ppX", bufs=1, space=PSUM) as ppX,
            tc.tile_pool(name="ppS", bufs=1, space=PSUM) as ppS,
            tc.tile_pool(name="ppT", bufs=2, space=PSUM) as ppT,
        ):
            # ---- inputs (DMA order = need order, dc-chunked so the first
            # projection matmul starts after ~1/4 of the input bytes) ----
            d1 = [constp.tile([128, 256 + QW], fp16, name=f"d1_{dc}")
                  for dc in range(NDC)]
            d2 = [constp.tile([128, 256 + LPT], fp16, name=f"d2_{dc}")
                  for dc in range(NDC)]
            for dc in range(NDC):
                nc.sync.dma_start(d1[dc][:], d1_ds[dc].ap())
            for dc in range(NDC):
                nc.sync.dma_start(d2[dc][:], d2_ds[dc].ap())
            wv = constp.tile([128, NHC], f32)
            nc.sync.dma_start(wv[:], wv_d.ap())
            # d3 (values+mask, 33% of input bytes, first needed ~10us later)
            # is deferred below via a WAW dep so it doesn't steal DMA
            # bandwidth from the projection inputs.
            d3 = constp.tile([128, 4 * DV + LPT], fp16)
            vals = d3[:, :4 * DV].rearrange("p (c v) -> p c v", c=4)
            maskv = d3[:, 4 * DV:]

            identf = constp.tile([128, 128], f32)
            masks.make_identity(nc, identf[:])
            ident_bf = constp.tile([128, 128], bf16)
            nc.vector.tensor_copy(ident_bf[:], identf[:])
            ones16 = constp.tile([128, 128], fp16)
            nc.vector.memset(ones16[:], 1.0)
            # dummy ops on 1 col, overlapped with the input DMA: force the
            # trig ACT table load, the GpSimd ucode IRAM load, and the custom
            # DVE table install to happen NOW rather than on first real use.
            dum = constp.tile([128, 1], fp16)
            nc.scalar.activation(dum[:], ones16[:, 0:1], AF.Sin, scale=0.1)
            dumg = constp.tile([128, 1], fp16)
            nc.gpsimd.tensor_scalar(dumg[:], ones16[:, 0:1], 1.0, 1.0,
                                    ALU.mult, ALU.mult)
            dumv = constp.tile([128, 1], fp16)
            nc.vector._custom_dve(RR, out=dumv[:], in0=ones16[:, 0:1],
                                  s0=0.1, s1=0.0, imm2=MAGIC)

            RLAST = len(SLOTS) - 1

            def norm_phase(w, s):
                """sin(wx+s) = sign*sin(wx+s'), |s'| <= pi/2, via s' = s-k*pi.
                The direct (non-RR) slot needs |w|*4.5 + |s'| within the Sin
                ACT's accurate range (~<=4.0 incl the saturation cubic)."""
                k = int(np.floor(s / np.pi + 0.5))
                s2 = s - k * np.pi
                return s2, (-1.0 if k % 2 else 1.0)

            # slot0 is the low-frequency slot: its args fit the Sin domain
            # directly, so it skips range reduction entirely (no V work) and
            # its atoms are computed straight off the projection PSUM.
            wq0, sq0, wk0, sk0, c0 = SLOTS[0]
            sq0n, sgq = norm_phase(wq0, sq0)
            sk0n, sgk = norm_phase(wk0, sk0)
            assert abs(wq0) * 4.6 + abs(sq0n) < 4.0
            assert abs(wk0) * 4.6 + abs(sk0n) < 4.0
            c0 = c0 * sgq * sgk
            bq0 = constp.tile([128, 1], f32, name="bq0")
            nc.vector.memset(bq0[:], float(sq0n))
            bk0 = constp.tile([128, 1], f32, name="bk0")
            nc.vector.memset(bk0[:], float(sk0n))

            # ---- projections into PSUM. Separate q/k tiles; consumers of
            # each are emitted immediately after its closing MM so their
            # release semaphores are as early as possible.
            xcpq = ppX.tile([128, NHC, QW], f32, tag="xcpq")
            xcpk = ppX.tile([128, NHC, 512], f32, tag="xcpk")
            for dc in range(NDC):
                for hc in range(NHC):
                    nc.tensor.matmul(
                        xcpq[:, hc, :],
                        d1[dc][:, 128 * hc:128 * (hc + 1)],
                        d1[dc][:, 256:256 + QW],
                        start=(dc == 0), stop=(dc == NDC - 1),
                    )

            # slot1's q-side RR (V) + direct slot0 q-atoms (S): released on
            # the q projections alone, overlapping the k projections. The
            # k-side RRs then interleave (q_r, k_r) so u1k — which gates the
            # whole S sin chain — lands as early as possible.
            utiles = {}
            for r in (1,):
                utiles[r] = uwp.tile([128, NHC, W], fp16, tag="u",
                                     name=f"u{r}")
                nc.vector._custom_dve(
                    RR, out=utiles[r][:, :, 0:QW], in0=xcpq[:, :, :],
                    s0=SLOTS[r][0] / TWO_PI, s1=SLOTS[r][1] / TWO_PI,
                    imm2=MAGIC)
            atom0 = atwp.tile([128, NHC, W], fp16, tag="at", name="at0")
            nc.scalar.activation(atom0[:, :, 0:QW], xcpq[:, :, :],
                                 AF.Sin, scale=float(wq0), bias=bq0[:])

            for dc in range(NDC):
                for hc in range(NHC):
                    nc.tensor.matmul(
                        xcpk[:, hc, 0:LPT],
                        d2[dc][:, 128 * hc:128 * (hc + 1)],
                        d2[dc][:, 256:256 + LPT],
                        start=(dc == 0), stop=(dc == NDC - 1),
                    )

            sc = [ppS.tile([128, 2, LPs[s]], f32, tag=f"sc{s}", name=f"sc{s}")
                  for s in (0, 1)]

            # direct slot0 k-atoms + fold + its MM group (scores start)
            atomk0 = atwp.tile([128, NHC, LPT], fp16, tag="atk0", name="atk0")
            nc.scalar.activation(atomk0[:], xcpk[:, :, 0:LPT],
                                 AF.Sin, scale=float(wk0), bias=bk0[:])
            vt0 = vtwp.tile([128, NHC, LPT], fp16, tag="vt", name="vt0")
            for hc in range(NHC):
                nc.gpsimd.tensor_scalar(
                    vt0[:, hc], atomk0[:, hc], wv[:, hc:hc + 1],
                    float(c0), ALU.mult, ALU.mult)
            for s in (0, 1):
                for qc in range(2):
                    for hc in range(NHC):
                        nc.tensor.matmul(
                            sc[s][:, qc, :],
                            atom0[:, hc, 256 * s + 128 * qc:
                                  256 * s + 128 * qc + 128],
                            vt0[:, hc, OFS[s]:OFS[s] + LPs[s]],
                            start=(qc == 0 and hc == 0), stop=False,
                        )

            # ---- RR basis slots 1..RLAST. RR ops read the projection PSUM
            # directly (custom DVE runs 1x either way). The mask pseudo-slot
            # interleaves into the last slot's PE stream.
            for r in range(1, len(SLOTS)):
                wq, sq, wk, sk, c = SLOTS[r]
                u = utiles.get(r)
                if u is None:
                    u = uwp.tile([128, NHC, W], fp16, tag="u", name=f"u{r}")
                    nc.vector._custom_dve(
                        RR, out=u[:, :, 0:QW], in0=xcpq[:, :, :],
                        s0=wq / TWO_PI, s1=sq / TWO_PI, imm2=MAGIC)
                nc.vector._custom_dve(
                    RR, out=u[:, :, QW:W], in0=xcpk[:, :, 0:LPT],
                    s0=wk / TWO_PI, s1=sk / TWO_PI, imm2=MAGIC)
                if r == 1:
                    # release the deferred values+mask DMA once slot1's
                    # RRs are done (projection inputs finished streaming):
                    # GpSimd copies one element of u into d3's corner
                    # (RAW on u + WAW on d3 gates the DMA trigger).
                    nc.gpsimd.tensor_scalar(d3[0:1, 0:1], u[0:1, 0, 0:1],
                                            1.0, None, ALU.mult)
                    nc.sync.dma_start(d3[:], d3_d.ap())
                atom = atwp.tile([128, NHC, W], fp16, tag="at", name=f"at{r}")
                vt = vtwp.tile([128, NHC, LPT], fp16, tag="vt", name=f"vt{r}")
                if r == RLAST:
                    # split the sins q-first (the MMs' lhsT is ready early),
                    # k-side into its OWN tile; folds on the now-idle V. V's
                    # sem wait releases one S-op late, so a 1-col dummy S op
                    # right after the k-sin makes the folds release at the
                    # k-sin's end instead of stalling to the next real ACT.
                    atomk = atwp.tile([128, NHC, LPT], fp16, tag="atk",
                                      name="atk")
                    nc.scalar.activation(atom[:, :, 0:QW], u[:, :, 0:QW],
                                         AF.Sin, scale=TWO_PI)
                    nc.scalar.activation(atomk[:], u[:, :, QW:W],
                                         AF.Sin, scale=TWO_PI)
                    nc.scalar.activation(dum[:], ones16[:, 0:1],
                                         AF.Sin, scale=0.1)
                    for hc in range(NHC):
                        nc.vector.tensor_scalar(
                            vt[:, hc], atomk[:, hc], wv[:, hc:hc + 1],
                            float(c), ALU.mult, ALU.mult)
                else:
                    atomk = atom[:, :, QW:W]
                    nc.scalar.activation(
                        atom[:].rearrange("p a b -> p (a b)"),
                        u[:].rearrange("p a b -> p (a b)"),
                        AF.Sin, scale=TWO_PI)
                    for hc in range(NHC):
                        # mid-phase folds ride the otherwise-idle GpSimd
                        nc.gpsimd.tensor_scalar(
                            vt[:, hc], atomk[:, hc], wv[:, hc:hc + 1],
                            float(c), ALU.mult, ALU.mult)
                for s in (0, 1):
                    for qc in range(2):
                        for hc in range(NHC):
                            nc.tensor.matmul(
                                sc[s][:, qc, :],
                                atom[:, hc, 256 * s + 128 * qc:
                                     256 * s + 128 * qc + 128],
                                vt[:, hc, OFS[s]:OFS[s] + LPs[s]],
                                start=False, stop=False,
                            )
                        if r == RLAST:
                            nc.tensor.matmul(
                                sc[s][:, qc, :], ones16[:],
                                maskv[:, OFS[s]:OFS[s] + LPs[s]],
                                start=False, stop=True,
                            )

            # ---- softmax + attn@V epilogue (s streams interleaved) ----
            p_ts, rsums, rinvs, pTs, outps = {}, {}, {}, {}, {}
            # exp on S WITHOUT the accumulator reads (4x182ns off the S
            # chain); row sums on the otherwise-idle V via tensor_reduce.
            for s in (0, 1):
                p_ts[s] = epip.tile([128, 2, LPs[s]], bf16, tag=f"pt{s}",
                                    name=f"pt{s}")
                rsums[s] = epip.tile([128, 2], f32, tag=f"rs{s}", name=f"rs{s}")
                for qc in range(2):
                    nc.scalar.activation(
                        p_ts[s][:, qc, :], sc[s][:, qc, :], AF.Exp)
                    nc.vector.tensor_reduce(
                        rsums[s][:, qc:qc + 1], p_ts[s][:, qc, :],
                        mybir.AxisListType.X, ALU.add)
            for s in (0, 1):
                rinvs[s] = epip.tile([128, 2], f32, tag=f"ri{s}", name=f"ri{s}")
                nc.vector.reciprocal(rinvs[s][:], rsums[s][:])
            # per-s: transposes+copies then attn MMs, so s0's attn does not
            # sit behind s1's transposes in the in-order PE stream
            ncp = 0
            for s in (0, 1):
                pTs[s] = epip.tile([128, NKC[s], 2, 128], bf16, tag=f"pT{s}",
                                   name=f"pT{s}")
                for qc in range(2):
                    for kc in range(NKC[s]):
                        kw_ = KCW[s][kc]
                        tp = ppT.tile([128, 128], bf16, tag="tp")
                        nc.tensor.transpose(
                            tp[:kw_, :128],
                            p_ts[s][:, qc, 128 * kc:128 * kc + kw_],
                            ident_bf[:, :])
                        if ncp % 2 == 0:
                            nc.vector.tensor_copy(pTs[s][:kw_, kc, qc],
                                                  tp[:kw_, :128])
                        else:
                            nc.scalar.copy(pTs[s][:kw_, kc, qc],
                                           tp[:kw_, :128])
                        ncp += 1
                outp = ppS.tile([128, 2, DV], f32, tag=f"sc{s}", name=f"op{s}")
                outps[s] = outp
                for qc in range(2):
                    # separate accumulation group per qc so qc0's output
                    # scale overlaps qc1's matmuls
                    for kc in range(NKC[s]):
                        kw_ = KCW[s][kc]
                        nc.tensor.matmul(
                            outp[:, qc, :], pTs[s][:kw_, kc, qc],
                            vals[:kw_, 2 * s + kc, :],
                            start=(kc == 0),
                            stop=(kc == NKC[s] - 1),
                        )
            # s0: one merged DMA. s1 (the tail): per-qc tiles + DMAs so the
            # first half ships while the second half's scale still runs.
            out_sb = epip.tile([128, 2, DV], bf16, tag="ob0", name="ob0")
            nc.vector.tensor_scalar(
                out_sb[:, 0, :], outps[0][:, 0, :],
                rinvs[0][:, 0:1], None, ALU.mult)
            nc.scalar.mul(out_sb[:, 1, :], outps[0][:, 1, :],
                          rinvs[0][:, 1:2])
            nc.sync.dma_start(
                out_d.ap()[0].rearrange("(c p) v -> p c v", c=2),
                out_sb[:])
            for qc in range(2):
                ob = epip.tile([128, DV], bf16, tag=f"ob1{qc}",
                               name=f"ob1{qc}")
                if qc == 0:
                    nc.vector.tensor_scalar(
                        ob[:], outps[1][:, 0, :], rinvs[1][:, 0:1],
                        None, ALU.mult)
                else:
                    nc.scalar.mul(ob[:], outps[1][:, 1, :], rinvs[1][:, 1:2])
                nc.sync.dma_start(
                    out_d.ap()[1, 128 * qc:128 * (qc + 1), :], ob[:])

    nc.compile()
    return nc


_GRAPH_CACHE = {}


def _get_graph(LP0, LP1):
    key = (LP0, LP1)
    if key not in _GRAPH_CACHE:
        _GRAPH_CACHE[key] = _build_graph(LP0, LP1)
    return _GRAPH_CACHE[key]


def kernel(queries, keys, values, valid_lens, W_q, W_k, w_v):
    from concourse import bass_utils

    queries = np.asarray(queries, dtype=np.float32)
    keys = np.asarray(keys, dtype=np.float32)
    values = np.asarray(values, dtype=np.float32)
    W_q = np.asarray(W_q, dtype=np.float32)
    W_k = np.asarray(W_k, dtype=np.float32)
    w_v = np.asarray(w_v, dtype=np.float32)
    vl = np.asarray(valid_lens).astype(np.int64)

    B = queries.shape[0]
    assert B == 2 * _NCORES

    # pair batches: sort desc, pair i with (B-1-i)
    order = np.argsort(-vl, kind="stable")
    pairs = [(int(order[i]), int(order[B - 1 - i])) for i in range(_NCORES)]
    lv = lambda b: int(min(NQ, max(1, vl[b])))
    LP0 = max(-(-lv(b0) // 8) * 8 for b0, b1 in pairs)
    LP1 = max(-(-lv(b1) // 8) * 8 for b0, b1 in pairs)
    LPT = LP0 + LP1
    QW = 2 * NQ

    nc = _get_graph(LP0, LP1)

    def t128(x, rows):  # [rows, 256] -> [128, NDC, rows]
        xt = np.ascontiguousarray(x[:rows].T)          # [256, rows]
        return xt.reshape(NDC, 128, rows).transpose(1, 0, 2)

    Wq_c = W_q.reshape(NDC, 128, H).transpose(1, 0, 2)   # [128, NDC, H]
    Wk_c = W_k.reshape(NDC, 128, H).transpose(1, 0, 2)
    wvp = np.ascontiguousarray(w_v.reshape(NHC, 128).T).astype(np.float32)

    in_maps = []
    for b0, b1 in pairs:
        d1 = np.empty((128, NDC, 256 + QW), np.float32)
        d1[:, :, :256] = Wq_c
        d1[:, :, 256:512] = t128(queries[b0], NQ)
        d1[:, :, 512:768] = t128(queries[b1], NQ)
        d2 = np.empty((128, NDC, 256 + LPT), np.float32)
        d2[:, :, :256] = Wk_c
        d2[:, :, 256:256 + LP0] = t128(keys[b0], LP0)
        d2[:, :, 256 + LP0:] = t128(keys[b1], LP1)
        d1h = np.ascontiguousarray(d1).astype(FP16)
        d2h = np.ascontiguousarray(d2).astype(FP16)
        d3 = np.zeros((128, 4 * DV + LPT), np.float32)
        for s, (b, LPs) in enumerate(((b0, LP0), (b1, LP1))):
            for kc in range(-(-LPs // 128)):
                kw_ = min(128, LPs - 128 * kc)
                d3[:kw_, DV * (2 * s + kc):DV * (2 * s + kc) + DV] = \
                    values[b, 128 * kc:128 * kc + kw_, :]
        mk = np.zeros((128, LPT), np.float32)
        mk[:, :LP0][:, np.arange(LP0) >= lv(b0)] = -8000.0
        mk[:, LP0:][:, np.arange(LP1) >= lv(b1)] = -8000.0
        d3[:, 4 * DV:] = mk
        im = {"vm": d3.astype(FP16), "wvp": wvp}
        for dc in range(NDC):
            im[f"wq_qT{dc}"] = np.ascontiguousarray(d1h[:, dc])
            im[f"wk_kT{dc}"] = np.ascontiguousarray(d2h[:, dc])
        in_maps.append(im)

    trace = os.environ.get("BASS_KERNEL_TRACE") == "1"
    if trace:
        _register_ntff_hook()
    res = bass_utils.run_bass_kernel_spmd(
        nc, in_maps, core_ids=list(range(_NCORES)), trace=trace)
    kernel.last_results = res

    out = np.empty((B, NQ, DV), dtype=np.float32)
    for j, (b0, b1) in enumerate(pairs):
        o = np.asarray(res.results[j]["out"]).astype(np.float32)
        out[b0] = o[0]
        out[b1] = o[1]
    return out
